# revision 36
# baseline (speedup 1.0000x reference)
"""Trainium2 Bass kernel for pooled-KV spatial attention.

Reference computation (per sample):
  q = Wq @ x            (8, 4096)
  k = maxpool2(Wk @ x)  (8, 1024)
  v = maxpool2(Wv @ x)  (32, 1024)
  w = softmax(q^T k over pooled axis)   (4096, 1024)
  o = v @ w^T -> Wo @ o                 (64, 4096)
  out = gamma * o + x

Sharding: batch 16 -> 2 samples per NeuronCore x 8 cores (pure data
parallel, no collectives).

Per-core kernel layout decisions:
  - all matmuls in bf16 (fp32 matmul is 4x slower per column on PE)
  - scores computed transposed: scT[m, n] = k^T q with K=8, x2 row-packed
    via tile_position so two (m,n) column chunks run concurrently
  - softmax denominators come free from the apply matmul by augmenting
    v^T with a ones column (row 32 / 96 of the accumulator = sum_m exp)
  - softmax normalization deferred to o (32 rows), broadcast of 1/sum via
    a small DRAM round-trip (DMA broadcast with step-0 leading dim)
  - residual add stays fully fp32
"""

import os
import sys
import time

if "/opt/trn_rl_repo" not in sys.path:
    sys.path.insert(0, "/opt/trn_rl_repo")

import ml_dtypes
import numpy as np

import concourse.bacc as bacc
import concourse.bass as bass
import concourse.tile as tile
from concourse import mybir
import concourse.bass_utils as bass_utils

BF16 = mybir.dt.bfloat16
F32 = mybir.dt.float32
AF = mybir.ActivationFunctionType
F32R = mybir.dt.float32r

B, C, H, W = 16, 64, 64, 64
HW = H * W                  # 4096
S4 = HW // 4                # 1024 pooled positions
CQ, CV = C // 8, C // 2     # 8, 32
NCORES = 8
BLOC = B // NCORES          # 2 samples per core
NJ = HW // 512              # 8 n-chunks of 512
NI = S4 // 128              # 8 m-chunks of 128

_CACHE = {}
LAST_EXEC_TIME_NS = None
LAST_TRACE = None


def _install_profile_hook():
    """Best-effort: register the axon NTFF profiling hook (antenv.axon_hooks
    shim) so trace=True yields exec_time_ns. Safe no-op on failure."""
    try:
        import types
        import antenv

        if "antenv.axon_hooks" in sys.modules:
            return
        holder = {"hook": None}
        mod = types.ModuleType("antenv.axon_hooks")
        mod.set_axon_ntff_profile_hook = lambda h: holder.__setitem__("hook", h)
        mod.get_axon_ntff_profile_hook = lambda: holder["hook"]
        sys.modules["antenv.axon_hooks"] = mod
        antenv.axon_hooks = mod
        from trn_agent_boot.trn_boot import _ntff_profile_via_ctypes

        hook = _ntff_profile_via_ctypes("/opt/axon/libaxon_pjrt.so")
        if hook is not None:
            mod.set_axon_ntff_profile_hook(hook)
        bass_utils.upload_artifacts = lambda tmpdir: tmpdir
    except Exception:
        pass


def build_nc():
    """Build the per-core Bass program (SPMD: same program on all 8 cores)."""
    nc = bacc.Bacc(
        "TRN2", target_bir_lowering=False, debug=False, enable_asserts=False
    )

    x_d = nc.dram_tensor("x", (BLOC, C, HW), F32, kind="ExternalInput").ap()
    wqkv_d = nc.dram_tensor("wqkvT", (C, 72), BF16, kind="ExternalInput").ap()
    wo_d = nc.dram_tensor("woT", (96, C), BF16, kind="ExternalInput").ap()
    id_d = nc.dram_tensor("ident", (CV, CV), BF16, kind="ExternalInput").ap()
    out_d = nc.dram_tensor("out", (BLOC, C, HW), F32, kind="ExternalOutput").ap()
    warm_d = nc.dram_tensor("wout", (1, 8), F32, kind="ExternalOutput").ap()
    ones2_d = nc.dram_tensor("ones2", (2, 128), F32, kind="ExternalInput").ap()

    from contextlib import ExitStack

    with tile.TileContext(nc) as tc, ExitStack() as ctx:
        ec = ctx.enter_context
        consts = ec(tc.tile_pool(name="consts", bufs=1))
        xpool = ec(tc.tile_pool(name="xpool", bufs=2))
        xbpool = ec(tc.tile_pool(name="xbpool", bufs=2))
        qpool = ec(tc.tile_pool(name="qpool", bufs=2))
        kpool = ec(tc.tile_pool(name="kpool", bufs=2))
        vkpool = ec(tc.tile_pool(name="vkpool", bufs=2))
        vaugpool = ec(tc.tile_pool(name="vaugpool", bufs=2))
        pooltmp = ec(tc.tile_pool(name="pooltmp", bufs=3))
        expp = ec(tc.tile_pool(name="expp", bufs=3))
        osbp = ec(tc.tile_pool(name="osbp", bufs=2))
        rcp = ec(tc.tile_pool(name="rcp", bufs=2))
        rbp = ec(tc.tile_pool(name="rbp", bufs=2))
        outp = ec(tc.tile_pool(name="outp", bufs=3))
        ps_sc = ec(tc.tile_pool(name="ps_sc", bufs=2, space="PSUM"))
        ps_acc = ec(tc.tile_pool(name="ps_acc", bufs=2, space="PSUM"))
        ps_misc = ec(tc.tile_pool(name="ps_misc", bufs=2, space="PSUM"))
        dramp = ec(tc.tile_pool(name="dramp", bufs=4, space="DRAM"))

        # ---- constants ----
        wqkv_sb = consts.tile([C, 72], BF16)
        nc.sync.dma_start(out=wqkv_sb[:], in_=wqkv_d)
        wo_sb = consts.tile([96, C], BF16)
        nc.sync.dma_start(out=wo_sb[:], in_=wo_d)
        # identity for the v^T transposes lives at partitions 32-63 to match
        # the v rows of the pooled tile
        id_sb = consts.tile([64, CV], BF16)
        nc.sync.dma_start(out=id_sb[32:64, :], in_=id_d)

        # ---- PE warm-up: ~5us of dense matmuls un-throttle the PE clock
        # (HAM releases 1.2 -> 2.4 GHz only after ~3.4us of sustained work)
        wrma = consts.tile([128, 128], BF16)
        nc.vector.memset(wrma[:], 0.001)
        wrm = consts.tile([128, 512], BF16)
        nc.vector.memset(wrm[:], 0.001)
        wps = ps_misc.tile([128, 512], F32, tag="m")
        NWARM = 9
        for w in range(NWARM):
            nc.tensor.matmul(
                wps[:], wrma[:], wrm[:],
                start=(w == 0), stop=(w == NWARM - 1),
            )
        wsb = consts.tile([1, 8], F32)
        nc.vector.tensor_copy(wsb[:], wps[0:1, 0:8])
        nc.sync.dma_start(out=warm_d, in_=wsb[:])

        # ones rows for the K=2 broadcast matmul: out[p,:] = rhs[32,:] for
        # p<64 else rhs[33,:]
        ones2_sb = consts.tile([34, 128], F32)
        nc.sync.dma_start(out=ones2_sb[32:34, :], in_=ones2_d)

        st = {}

        def emit_sample_head(b):
            x_t = xpool.tile([C, HW], F32, tag="x", name=f"x_{b}")
            nc.sync.dma_start(out=x_t[:], in_=x_d[b])
            xb_t = xbpool.tile([C, HW], BF16, tag="xb", name=f"xb_{b}")
            nc.vector.tensor_copy(xb_t[:], x_t[:])
            # kb/qb are zero-padded to K=128 so the scores matmuls use the
            # full contraction dim (the HAM activity monitor only un-throttles
            # the PE clock for high-K matmuls).  Each chunk is its own tile so
            # attention chunk (jj, i) only depends on its projection chunks.
            qbs_j, kbs_i, vaugs_i = [], [], []
            for j in range(NJ):
                qbj = qpool.tile([128, 512], BF16, tag=f"qb{j}", name=f"qb{j}_{b}")
                nc.vector.memset(qbj[:], 0.0)
                qbs_j.append(qbj)
            for i in range(NI):
                kbi = kpool.tile([128, 128], BF16, tag=f"kb{i}", name=f"kb{i}_{b}")
                nc.vector.memset(kbi[:], 0.0)
                kbs_i.append(kbi)
                # cols 33-63 of each vaug block are zero-pad (apply M=64),
                # col 32 is the ones column for the softmax denominators
                vai = vaugpool.tile([128, 64], BF16, tag=f"va{i}", name=f"va{i}_{b}")
                nc.vector.memset(vai[:], 0.0)
                nc.vector.memset(vai[:, 32:33], 1.0)
                vaugs_i.append(vai)
            vkb = vkpool.tile([64, S4], BF16, tag="vkb", name=f"vkb_{b}")
            st[b] = dict(x=x_t, xb=xb_t, qb=qbs_j, kb=kbs_i, va=vaugs_i, vkb=vkb)

        def emit_proj_chunk(b, j):
            """Fused k/v/q projection + 2x2 maxpool + v^T for one 512-chunk.
            Partition layout (SBUF window rule): k rows 0-7, v rows 32-63,
            q rows 64-71."""
            s = st[b]
            js = slice(512 * j, 512 * (j + 1))
            proj = ps_misc.tile([128, 512], F32, tag="m", name=f"proj{j}_{b}")
            nc.tensor.matmul(
                proj[0:72, :], wqkv_sb[:, 0:72], s["xb"][:, js],
                start=True, stop=True,
            )
            pcp = pooltmp.tile([72, 512], BF16, tag="pcp", name=f"pcp{j}_{b}")
            nc.vector.tensor_copy(pcp[0:72, :], proj[0:72, :])
            p4 = pcp[0:64, :].rearrange(
                "p (h w2 wp) -> p h w2 wp", h=8, w2=32, wp=2
            )
            st1 = pooltmp.tile([64, 256], BF16, tag="st1", name=f"st1_{j}_{b}")
            nc.vector.tensor_max(st1[0:64, :], p4[:, :, :, 0], p4[:, :, :, 1])
            s4 = st1[0:64, :].rearrange(
                "p (h2 hp w2) -> p h2 hp w2", h2=4, hp=2, w2=32
            )
            ms = slice(128 * j, 128 * (j + 1))
            vkb = s["vkb"]
            nc.vector.tensor_max(vkb[0:64, ms], s4[:, :, 0, :], s4[:, :, 1, :])
            nc.sync.dma_start(out=s["kb"][j][0:8, :], in_=vkb[0:8, ms])
            nc.sync.dma_start(out=s["qb"][j][0:8, :], in_=pcp[64:72, :])
            vt = ps_misc.tile([128, CV], BF16, tag="m", name=f"vt{j}_{b}")
            nc.tensor.transpose(vt[:], vkb[32:64, ms], id_sb[32:64, :])
            nc.vector.tensor_copy(s["va"][j][:, 0:32], vt[:])

        def emit_attn_pair(b, jj):
            """Attention for a pair of 512-wide n-chunks (j = 2jj, 2jj+1)."""
            s = st[b]
            o_t = ps_acc.tile([128, 512], F32, tag="o", name=f"o_{jj}_{b}")
            for i in range(NI):
                sc = ps_sc.tile([128, 1024], F32, tag="sc", name=f"sc{jj}_{i}_{b}")
                nc.tensor.matmul(
                    sc[:, 0:512], s["kb"][i][:], s["qb"][2 * jj][:],
                    start=True, stop=True,
                )
                nc.tensor.matmul(
                    sc[:, 512:1024], s["kb"][i][:], s["qb"][2 * jj + 1][:],
                    start=True, stop=True,
                )
                ex = expp.tile([128, 1024], BF16, tag="ex", name=f"ex{jj}_{i}_{b}")
                nc.scalar.activation(ex[:], sc[:], AF.Exp)
                va = s["va"][i][:]
                nc.tensor.matmul(
                    o_t[0:64, :], va, ex[:, 0:512],
                    start=(i == 0), stop=(i == NI - 1),
                    tile_position=(0, 0), skip_group_check=True,
                )
                nc.tensor.matmul(
                    o_t[64:128, :], va, ex[:, 512:1024],
                    start=(i == 0), stop=(i == NI - 1),
                    tile_position=(0, 64), skip_group_check=True,
                )
            # copy the accumulator to SBUF (frees the PSUM bank immediately),
            # reciprocal on a narrow [128, 8] layout, then broadcast the two
            # recip rows across partitions with a K=2 ones-matmul
            o_sb = rcp.tile([128, 512], F32, tag="o_sb", name=f"osb{jj}_{b}")
            nc.vector.tensor_copy(o_sb[:], o_t[:])
            s16 = rcp.tile([128, 8], F32, tag="s16", name=f"s16_{jj}_{b}")
            nc.sync.dma_start(out=s16[0:64, :], in_=o_sb[32:33, :])
            nc.sync.dma_start(out=s16[64:128, :], in_=o_sb[96:97, :])
            rc16 = rcp.tile([128, 8], F32, tag="rc16", name=f"rc16_{jj}_{b}")
            nc.vector.reciprocal(rc16[:], s16[:])
            rcrow = rcp.tile([34, 512], F32, tag="rcrow", name=f"rcr{jj}_{b}")
            nc.sync.dma_start(out=rcrow[32:33, :], in_=rc16[0:64, :])
            nc.sync.dma_start(out=rcrow[33:34, :], in_=rc16[64:128, :])
            rb_ps = ps_misc.tile([128, 512], F32, tag="m", name=f"rb{jj}_{b}")
            nc.tensor.matmul(
                rb_ps[:], ones2_sb[32:34, :], rcrow[32:34, :],
                start=True, stop=True, tile_position=(32, 0),
            )
            on_t = osbp.tile([128, 512], BF16, tag="on", name=f"on{jj}_{b}")
            nc.vector.tensor_mul(on_t[:], o_sb[:], rb_ps[:])
            # output projection + residual, one 512-chunk per group
            out_t = outp.tile([C, 1024], F32, tag="out", name=f"out{jj}_{b}")
            for g in range(2):
                j = 2 * jj + g
                js = slice(512 * j, 512 * (j + 1))
                wo_t = ps_misc.tile([C, 512], F32, tag="m", name=f"wo{j}_{b}")
                nc.tensor.matmul(
                    wo_t[:],
                    wo_sb[64 * g : 64 * g + CV, :],
                    on_t[64 * g : 64 * g + CV, :],
                    start=True, stop=True, tile_position=(64 * g, 0),
                )
                nc.vector.tensor_add(
                    out_t[:, 512 * g : 512 * g + 512], wo_t[:], s["x"][:, js]
                )
            nc.sync.dma_start(
                out=out_d[b][:, 1024 * jj : 1024 * (jj + 1)], in_=out_t[:]
            )

        # emission order interleaves sample 1's projections into sample 0's
        # attention so the in-order PE stream never sits behind a cold phase
        emit_sample_head(0)
        for j in range(NJ):
            emit_proj_chunk(0, j)
        emit_sample_head(1)
        for jj in range(NJ // 2):
            emit_attn_pair(0, jj)
            emit_proj_chunk(1, 2 * jj)
            emit_proj_chunk(1, 2 * jj + 1)
        for jj in range(NJ // 2):
            emit_attn_pair(1, jj)

    nc.compile()
    return nc


def _get_nc():
    if "nc" not in _CACHE:
        _install_profile_hook()
        _CACHE["nc"] = build_nc()
    return _CACHE["nc"]


def host_prep(x, Wq, Wk, Wv, Wo, gamma):
    x = np.asarray(x, dtype=np.float32)
    gamma_f = float(np.asarray(gamma, dtype=np.float32))
    wqkvT = np.zeros((C, 72), dtype=ml_dtypes.bfloat16)
    wqkvT[:, 0:8] = np.asarray(Wk).T.astype(ml_dtypes.bfloat16)
    wqkvT[:, 32:64] = np.asarray(Wv).T.astype(ml_dtypes.bfloat16)
    wqkvT[:, 64:72] = np.asarray(Wq).T.astype(ml_dtypes.bfloat16)
    woT = np.zeros((96, C), dtype=ml_dtypes.bfloat16)
    wog = (gamma_f * np.asarray(Wo, dtype=np.float32)).T.astype(ml_dtypes.bfloat16)
    woT[0:32] = wog
    woT[64:96] = wog
    ident = np.eye(CV, dtype=ml_dtypes.bfloat16)
    ones2 = np.zeros((2, 128), dtype=np.float32)
    ones2[0, 0:64] = 1.0
    ones2[1, 64:128] = 1.0

    xr = x.reshape(B, C, HW)
    return [
        {
            "x": np.ascontiguousarray(xr[BLOC * i : BLOC * (i + 1)]),
            "wqkvT": wqkvT,
            "woT": woT,
            "ident": ident,
            "ones2": ones2,
        }
        for i in range(NCORES)
    ]


def kernel(x, Wq, Wk, Wv, Wo, gamma):
    global LAST_EXEC_TIME_NS, LAST_TRACE
    nc = _get_nc()
    in_maps = host_prep(x, Wq, Wk, Wv, Wo, gamma)

    trace = bool(int(os.environ.get("BASS_KERNEL_TRACE", "0")))
    kwargs = {}
    if trace:
        kwargs["tmpdir"] = os.environ.get("BASS_KERNEL_TMPDIR") or None
    res = bass_utils.run_bass_kernel_spmd(
        nc, in_maps, core_ids=list(range(NCORES)), trace=trace, **kwargs
    )
    LAST_EXEC_TIME_NS = res.exec_time_ns
    LAST_TRACE = res.instructions_and_trace[1] if res.instructions_and_trace else None
    out = np.concatenate([res.results[i]["out"] for i in range(NCORES)], axis=0)
    return np.ascontiguousarray(out.reshape(B, C, H, W).astype(np.float32))


if __name__ == "__main__":
    xs = np.random.randn(B, C, H, W).astype(np.float32)
    o = kernel(
        xs,
        0.05 * np.random.randn(8, 64).astype(np.float32),
        0.05 * np.random.randn(8, 64).astype(np.float32),
        0.05 * np.random.randn(32, 64).astype(np.float32),
        0.05 * np.random.randn(64, 32).astype(np.float32),
        np.float32(0.5),
    )
    print(o.shape, o.dtype, LAST_EXEC_TIME_NS)


# revision 37
# speedup vs baseline: 1.0646x; 1.0646x over previous
"""Trainium2 Bass kernel for pooled-KV spatial attention.

Reference computation (per sample):
  q = Wq @ x            (8, 4096)
  k = maxpool2(Wk @ x)  (8, 1024)
  v = maxpool2(Wv @ x)  (32, 1024)
  w = softmax(q^T k over pooled axis)   (4096, 1024)
  o = v @ w^T -> Wo @ o                 (64, 4096)
  out = gamma * o + x

Sharding: batch 16 -> 2 samples per NeuronCore x 8 cores (pure data
parallel, no collectives).

Per-core kernel layout decisions:
  - all matmuls in bf16 (fp32 matmul is 4x slower per column on PE)
  - scores computed transposed: scT[m, n] = k^T q with K=8, x2 row-packed
    via tile_position so two (m,n) column chunks run concurrently
  - softmax denominators come free from the apply matmul by augmenting
    v^T with a ones column (row 32 / 96 of the accumulator = sum_m exp)
  - softmax normalization deferred to o (32 rows), broadcast of 1/sum via
    a small DRAM round-trip (DMA broadcast with step-0 leading dim)
  - residual add stays fully fp32
"""

import os
import sys
import time

if "/opt/trn_rl_repo" not in sys.path:
    sys.path.insert(0, "/opt/trn_rl_repo")

import ml_dtypes
import numpy as np

import concourse.bacc as bacc
import concourse.bass as bass
import concourse.tile as tile
from concourse import mybir
import concourse.bass_utils as bass_utils

BF16 = mybir.dt.bfloat16
F32 = mybir.dt.float32
AF = mybir.ActivationFunctionType
F32R = mybir.dt.float32r

B, C, H, W = 16, 64, 64, 64
HW = H * W                  # 4096
S4 = HW // 4                # 1024 pooled positions
CQ, CV = C // 8, C // 2     # 8, 32
NCORES = 8
BLOC = B // NCORES          # 2 samples per core
NJ = HW // 512              # 8 n-chunks of 512
NI = S4 // 128              # 8 m-chunks of 128

_CACHE = {}
LAST_EXEC_TIME_NS = None
LAST_TRACE = None


def _install_profile_hook():
    """Best-effort: register the axon NTFF profiling hook (antenv.axon_hooks
    shim) so trace=True yields exec_time_ns. Safe no-op on failure."""
    try:
        import types
        import antenv

        if "antenv.axon_hooks" in sys.modules:
            return
        holder = {"hook": None}
        mod = types.ModuleType("antenv.axon_hooks")
        mod.set_axon_ntff_profile_hook = lambda h: holder.__setitem__("hook", h)
        mod.get_axon_ntff_profile_hook = lambda: holder["hook"]
        sys.modules["antenv.axon_hooks"] = mod
        antenv.axon_hooks = mod
        from trn_agent_boot.trn_boot import _ntff_profile_via_ctypes

        hook = _ntff_profile_via_ctypes("/opt/axon/libaxon_pjrt.so")
        if hook is not None:
            mod.set_axon_ntff_profile_hook(hook)
        bass_utils.upload_artifacts = lambda tmpdir: tmpdir
    except Exception:
        pass


def build_nc():
    """Build the per-core Bass program (SPMD: same program on all 8 cores)."""
    nc = bacc.Bacc(
        "TRN2", target_bir_lowering=False, debug=False, enable_asserts=False
    )

    x_d = nc.dram_tensor("x", (BLOC, C, HW), F32, kind="ExternalInput").ap()
    wqkv_d = nc.dram_tensor("wqkvT", (C, 72), BF16, kind="ExternalInput").ap()
    wo_d = nc.dram_tensor("woT", (96, C), BF16, kind="ExternalInput").ap()
    id_d = nc.dram_tensor("ident", (CV, CV), BF16, kind="ExternalInput").ap()
    out_d = nc.dram_tensor("out", (BLOC, C, HW), F32, kind="ExternalOutput").ap()
    warm_d = nc.dram_tensor("wout", (1, 8), F32, kind="ExternalOutput").ap()
    ones2_d = nc.dram_tensor("ones2", (2, 128), F32, kind="ExternalInput").ap()

    from contextlib import ExitStack

    with tile.TileContext(nc) as tc, ExitStack() as ctx:
        ec = ctx.enter_context
        consts = ec(tc.tile_pool(name="consts", bufs=1))
        xpool = ec(tc.tile_pool(name="xpool", bufs=2))
        xbpool = ec(tc.tile_pool(name="xbpool", bufs=2))
        qpool = ec(tc.tile_pool(name="qpool", bufs=2))
        kpool = ec(tc.tile_pool(name="kpool", bufs=2))
        vkpool = ec(tc.tile_pool(name="vkpool", bufs=2))
        vaugpool = ec(tc.tile_pool(name="vaugpool", bufs=2))
        pooltmp = ec(tc.tile_pool(name="pooltmp", bufs=3))
        expp = ec(tc.tile_pool(name="expp", bufs=3))
        osbp = ec(tc.tile_pool(name="osbp", bufs=2))
        rcp = ec(tc.tile_pool(name="rcp", bufs=2))
        rbp = ec(tc.tile_pool(name="rbp", bufs=2))
        outp = ec(tc.tile_pool(name="outp", bufs=3))
        ps_sc = ec(tc.tile_pool(name="ps_sc", bufs=2, space="PSUM"))
        ps_acc = ec(tc.tile_pool(name="ps_acc", bufs=2, space="PSUM"))
        ps_misc = ec(tc.tile_pool(name="ps_misc", bufs=2, space="PSUM"))
        dramp = ec(tc.tile_pool(name="dramp", bufs=4, space="DRAM"))

        # ---- constants ----
        wqkv_sb = consts.tile([C, 72], BF16)
        nc.sync.dma_start(out=wqkv_sb[:], in_=wqkv_d)
        wo_sb = consts.tile([96, C], BF16)
        nc.sync.dma_start(out=wo_sb[:], in_=wo_d)
        # identity for the v^T transposes lives at partitions 32-63 to match
        # the v rows of the pooled tile
        id_sb = consts.tile([64, CV], BF16)
        nc.sync.dma_start(out=id_sb[32:64, :], in_=id_d)

        # ---- PE warm-up: ~5us of dense matmuls un-throttle the PE clock
        # (HAM releases 1.2 -> 2.4 GHz only after ~3.4us of sustained work)
        wrma = consts.tile([128, 128], BF16)
        nc.vector.memset(wrma[:], 0.001)
        wrm = consts.tile([128, 512], BF16)
        nc.vector.memset(wrm[:], 0.001)
        wps = ps_misc.tile([128, 512], F32, tag="m")
        NWARM = 9
        for w in range(NWARM):
            nc.tensor.matmul(
                wps[:], wrma[:], wrm[:],
                start=(w == 0), stop=(w == NWARM - 1),
            )
        wsb = consts.tile([1, 8], F32)
        nc.vector.tensor_copy(wsb[:], wps[0:1, 0:8])
        nc.sync.dma_start(out=warm_d, in_=wsb[:])

        # ones rows for the K=2 broadcast matmul: out[p,:] = rhs[32,:] for
        # p<64 else rhs[33,:]
        ones2_sb = consts.tile([34, 128], F32)
        nc.sync.dma_start(out=ones2_sb[32:34, :], in_=ones2_d)

        st = {}

        def emit_sample_head(b):
            x_t = xpool.tile([C, HW], F32, tag="x", name=f"x_{b}")
            nc.sync.dma_start(out=x_t[:], in_=x_d[b])
            xb_t = xbpool.tile([C, HW], BF16, tag="xb", name=f"xb_{b}")
            if b == 0:
                nc.scalar.copy(xb_t[:], x_t[:])
            else:
                nc.vector.tensor_copy(xb_t[:], x_t[:])
            # kb/qb are zero-padded to K=128 so the scores matmuls use the
            # full contraction dim (the HAM activity monitor only un-throttles
            # the PE clock for high-K matmuls).  Each chunk is its own tile so
            # attention chunk (jj, i) only depends on its projection chunks.
            qbs_j, kbs_i, vaugs_i = [], [], []
            for j in range(NJ):
                qbj = qpool.tile([128, 512], BF16, tag=f"qb{j}", name=f"qb{j}_{b}")
                nc.gpsimd.memset(qbj[:], 0.0)
                qbs_j.append(qbj)
            for i in range(NI):
                kbi = kpool.tile([128, 128], BF16, tag=f"kb{i}", name=f"kb{i}_{b}")
                nc.gpsimd.memset(kbi[:], 0.0)
                kbs_i.append(kbi)
                # cols 33-63 of each vaug block are zero-pad (apply M=64),
                # col 32 is the ones column for the softmax denominators
                vai = vaugpool.tile([128, 64], BF16, tag=f"va{i}", name=f"va{i}_{b}")
                nc.gpsimd.memset(vai[:], 0.0)
                nc.gpsimd.memset(vai[:, 32:33], 1.0)
                vaugs_i.append(vai)
            vkb = vkpool.tile([64, S4], BF16, tag="vkb", name=f"vkb_{b}")
            st[b] = dict(x=x_t, xb=xb_t, qb=qbs_j, kb=kbs_i, va=vaugs_i, vkb=vkb)

        def emit_proj_chunk(b, j):
            """Fused k/v/q projection + 2x2 maxpool + v^T for one 512-chunk.
            Partition layout (SBUF window rule): k rows 0-7, v rows 32-63,
            q rows 64-71."""
            s = st[b]
            js = slice(512 * j, 512 * (j + 1))
            proj = ps_misc.tile([128, 512], F32, tag="m", name=f"proj{j}_{b}")
            nc.tensor.matmul(
                proj[0:72, :], wqkv_sb[:, 0:72], s["xb"][:, js],
                start=True, stop=True,
            )
            pcp = pooltmp.tile([72, 512], BF16, tag="pcp", name=f"pcp{j}_{b}")
            if b == 0:
                nc.scalar.copy(pcp[0:72, :], proj[0:72, :])
            else:
                nc.vector.tensor_copy(pcp[0:72, :], proj[0:72, :])
            p4 = pcp[0:64, :].rearrange(
                "p (h w2 wp) -> p h w2 wp", h=8, w2=32, wp=2
            )
            st1 = pooltmp.tile([64, 256], BF16, tag="st1", name=f"st1_{j}_{b}")
            nc.vector.tensor_max(st1[0:64, :], p4[:, :, :, 0], p4[:, :, :, 1])
            s4 = st1[0:64, :].rearrange(
                "p (h2 hp w2) -> p h2 hp w2", h2=4, hp=2, w2=32
            )
            ms = slice(128 * j, 128 * (j + 1))
            vkb = s["vkb"]
            nc.vector.tensor_max(vkb[0:64, ms], s4[:, :, 0, :], s4[:, :, 1, :])
            nc.sync.dma_start(out=s["kb"][j][0:8, :], in_=vkb[0:8, ms])
            nc.sync.dma_start(out=s["qb"][j][0:8, :], in_=pcp[64:72, :])
            vt = ps_misc.tile([128, CV], BF16, tag="m", name=f"vt{j}_{b}")
            nc.tensor.transpose(vt[:], vkb[32:64, ms], id_sb[32:64, :])
            nc.vector.tensor_copy(s["va"][j][:, 0:32], vt[:])

        def emit_attn_pair(b, jj):
            """Attention for a pair of 512-wide n-chunks (j = 2jj, 2jj+1).
            Returns a closure emitting the PE tail (broadcast matmul, output
            projection, residual add, store) which the caller defers past the
            next pair's matmuls so the in-order PE stream never stalls on the
            normalization DMA chain."""
            s = st[b]
            o_t = ps_acc.tile([128, 512], F32, tag="o", name=f"o_{jj}_{b}")
            for i in range(NI):
                sc = ps_sc.tile([128, 1024], F32, tag="sc", name=f"sc{jj}_{i}_{b}")
                nc.tensor.matmul(
                    sc[:, 0:512], s["kb"][i][:], s["qb"][2 * jj][:],
                    start=True, stop=True,
                )
                nc.tensor.matmul(
                    sc[:, 512:1024], s["kb"][i][:], s["qb"][2 * jj + 1][:],
                    start=True, stop=True,
                )
                ex = expp.tile([128, 1024], BF16, tag="ex", name=f"ex{jj}_{i}_{b}")
                nc.scalar.activation(ex[:], sc[:], AF.Exp)
                va = s["va"][i][:]
                nc.tensor.matmul(
                    o_t[0:64, :], va, ex[:, 0:512],
                    start=(i == 0), stop=(i == NI - 1),
                    tile_position=(0, 0), skip_group_check=True,
                )
                nc.tensor.matmul(
                    o_t[64:128, :], va, ex[:, 512:1024],
                    start=(i == 0), stop=(i == NI - 1),
                    tile_position=(0, 64), skip_group_check=True,
                )
            # copy the accumulator to SBUF (frees the PSUM bank immediately),
            # reciprocal on a narrow [128, 8] layout; the DMA hops overlap the
            # next pair's matmuls
            o_sb = rcp.tile([128, 512], F32, tag="o_sb", name=f"osb{jj}_{b}")
            nc.vector.tensor_copy(o_sb[:], o_t[:])
            s16 = rcp.tile([128, 8], F32, tag="s16", name=f"s16_{jj}_{b}")
            nc.sync.dma_start(out=s16[0:64, :], in_=o_sb[32:33, :])
            nc.sync.dma_start(out=s16[64:128, :], in_=o_sb[96:97, :])
            rc16 = rcp.tile([128, 8], F32, tag="rc16", name=f"rc16_{jj}_{b}")
            nc.vector.reciprocal(rc16[:], s16[:])
            rcrow = rcp.tile([34, 512], F32, tag="rcrow", name=f"rcr{jj}_{b}")
            nc.sync.dma_start(out=rcrow[32:33, :], in_=rc16[0:64, :])
            nc.sync.dma_start(out=rcrow[33:34, :], in_=rc16[64:128, :])

            def tail():
                # broadcast the two recip rows across partitions (K=2
                # ones-matmul), normalize, project, add residual, store
                rb_ps = ps_misc.tile([128, 512], F32, tag="m", name=f"rb{jj}_{b}")
                nc.tensor.matmul(
                    rb_ps[:], ones2_sb[32:34, :], rcrow[32:34, :],
                    start=True, stop=True, tile_position=(32, 0),
                )
                on_t = osbp.tile([128, 512], BF16, tag="on", name=f"on{jj}_{b}")
                nc.vector.tensor_mul(on_t[:], o_sb[:], rb_ps[:])
                out_t = outp.tile([C, 1024], F32, tag="out", name=f"out{jj}_{b}")
                for g in range(2):
                    j = 2 * jj + g
                    js = slice(512 * j, 512 * (j + 1))
                    wo_t = ps_misc.tile([C, 512], F32, tag="m", name=f"wo{j}_{b}")
                    nc.tensor.matmul(
                        wo_t[:],
                        wo_sb[64 * g : 64 * g + CV, :],
                        on_t[64 * g : 64 * g + CV, :],
                        start=True, stop=True, tile_position=(64 * g, 0),
                    )
                    nc.vector.tensor_add(
                        out_t[:, 512 * g : 512 * g + 512], wo_t[:], s["x"][:, js]
                    )
                nc.sync.dma_start(
                    out=out_d[b][:, 1024 * jj : 1024 * (jj + 1)], in_=out_t[:]
                )

            return tail

        # emission order: sample-1 projection chunks and each pair's tail
        # are woven between attention pairs so the in-order PE stream always
        # has ready matmuls
        emit_sample_head(0)
        for j in range(NJ):
            emit_proj_chunk(0, j)
        emit_sample_head(1)
        b1_chunks = [[0, 1, 2], [3, 4, 5], [6, 7], []]
        pend = None
        for jj in range(NJ // 2):
            t = emit_attn_pair(0, jj)
            if pend is not None:
                pend()
            pend = t
            for j in b1_chunks[jj]:
                emit_proj_chunk(1, j)
        for jj in range(NJ // 2):
            t = emit_attn_pair(1, jj)
            if pend is not None:
                pend()
            pend = t
        pend()

    nc.compile()
    return nc


def _get_nc():
    if "nc" not in _CACHE:
        _install_profile_hook()
        _CACHE["nc"] = build_nc()
    return _CACHE["nc"]


def host_prep(x, Wq, Wk, Wv, Wo, gamma):
    x = np.asarray(x, dtype=np.float32)
    gamma_f = float(np.asarray(gamma, dtype=np.float32))
    wqkvT = np.zeros((C, 72), dtype=ml_dtypes.bfloat16)
    wqkvT[:, 0:8] = np.asarray(Wk).T.astype(ml_dtypes.bfloat16)
    wqkvT[:, 32:64] = np.asarray(Wv).T.astype(ml_dtypes.bfloat16)
    wqkvT[:, 64:72] = np.asarray(Wq).T.astype(ml_dtypes.bfloat16)
    woT = np.zeros((96, C), dtype=ml_dtypes.bfloat16)
    wog = (gamma_f * np.asarray(Wo, dtype=np.float32)).T.astype(ml_dtypes.bfloat16)
    woT[0:32] = wog
    woT[64:96] = wog
    ident = np.eye(CV, dtype=ml_dtypes.bfloat16)
    ones2 = np.zeros((2, 128), dtype=np.float32)
    ones2[0, 0:64] = 1.0
    ones2[1, 64:128] = 1.0

    xr = x.reshape(B, C, HW)
    return [
        {
            "x": np.ascontiguousarray(xr[BLOC * i : BLOC * (i + 1)]),
            "wqkvT": wqkvT,
            "woT": woT,
            "ident": ident,
            "ones2": ones2,
        }
        for i in range(NCORES)
    ]


def kernel(x, Wq, Wk, Wv, Wo, gamma):
    global LAST_EXEC_TIME_NS, LAST_TRACE
    nc = _get_nc()
    in_maps = host_prep(x, Wq, Wk, Wv, Wo, gamma)

    trace = bool(int(os.environ.get("BASS_KERNEL_TRACE", "0")))
    kwargs = {}
    if trace:
        kwargs["tmpdir"] = os.environ.get("BASS_KERNEL_TMPDIR") or None
    res = bass_utils.run_bass_kernel_spmd(
        nc, in_maps, core_ids=list(range(NCORES)), trace=trace, **kwargs
    )
    LAST_EXEC_TIME_NS = res.exec_time_ns
    LAST_TRACE = res.instructions_and_trace[1] if res.instructions_and_trace else None
    out = np.concatenate([res.results[i]["out"] for i in range(NCORES)], axis=0)
    return np.ascontiguousarray(out.reshape(B, C, H, W).astype(np.float32))


if __name__ == "__main__":
    xs = np.random.randn(B, C, H, W).astype(np.float32)
    o = kernel(
        xs,
        0.05 * np.random.randn(8, 64).astype(np.float32),
        0.05 * np.random.randn(8, 64).astype(np.float32),
        0.05 * np.random.randn(32, 64).astype(np.float32),
        0.05 * np.random.randn(64, 32).astype(np.float32),
        np.float32(0.5),
    )
    print(o.shape, o.dtype, LAST_EXEC_TIME_NS)
